# revision 7
# baseline (speedup 1.0000x reference)
"""Biaffine kernel for Trainium2, 8-core SPMD.

Math (reference):
    out[b,x,y,o] = bwn0 * sum_{i,j<=512} x1b[b,x,i] W_bil[o,i,j] x2b[b,y,j]
                 + bwn1 * (x1@W_lin[:512] [b,x,o] + x2@W_lin[512:] [b,y,o] + b_lin[o])
    with x1b/x2b = x append-ones, bwn = softmax(bw).

Decomposition (exact):
    out[b,x,y,o] = sum_{j<512} x2[b,y,j] * UT[b,o][j,x]      (step B, PE)
                 + D1[b,x,o] + D2[b,y,o]                      (rank-2, added on HOST)
    UT[b,o][j,x] = sum_{i<512} (bwn0*W_bil[o,i,j]) * x1[b,x,i]   (step A, PE)
    The device stream is pure 512-deep bf16 matmuls; the cheap rank-2 terms
    (x1@G+g0, x2@V) and the final broadcast-add run on the host, keeping the
    DVE to plain PSUM->SBUF casts that never gate the PE.

Sharding: tensor-parallel over O (128 output channels -> 16 per core).

Schedule notes:
  - PSUM tiles are 2-bank pairs ([128,1024] f32); each pair holds two 4-MM
    accumulation groups and is evicted by ONE 1024-wide DVE cast, halving
    the per-group eviction overhead (psA 2x2 + psB 2x2 = 8 banks).
  - Startup: X1 arrives in 4 it-chunks (2KB descriptors) and W0 in 4
    256KB chunks so the first matmul group only waits on ~512KB; all other
    loads (X2, W1, W2) are gated on BOTH X1 and W0 completing so they can't
    steal HBM bandwidth from the critical path (the v1 gate released them
    on X1 alone, which starved W0 for >10us).
  - 8 dummy matmuls on a zeroed scratch tile run during the startup DMA
    wait so the PE's HAM activity window is already warm when real work
    arrives.
  - OUT is written bf16 (host upcasts), halving output DMA traffic.
"""

import ml_dtypes
import numpy as np

import concourse.bass as bass
import concourse.mybir as mybir
import concourse.tile as tile
from concourse.bass_utils import run_bass_kernel_spmd

B, L, D, O = 4, 256, 512, 128
BL = B * L
N_CORES = 8
O_LOC = O // N_CORES          # 16 output channels per core
N_O2 = O_LOC // 2             # 8 o-pairs per core
F32 = mybir.dt.float32
BF16 = mybir.dt.bfloat16


# --------------------------------------------------------------------------
# Workaround: this container's walrus build accepts only ONE sync wait per
# instruction ("Too many sync wait commands").  Tile's wait assignment can
# attach several.  Post-pass: hoist extra waits onto InstEventSemaphore
# wait-carriers inserted immediately before the instruction on the same
# engine stream (same stall point, identical semantics).
_WS_CTR = [0]


def _split_multi_waits(nc):
    for f in nc.m.functions:
        for blk in f.blocks:
            insts = blk.instructions
            new = []
            changed = False
            for inst in insts:
                si = inst.sync_info
                waits = list(si.on_wait) if (si and si.on_wait) else []
                if len(waits) > 1:
                    for w in waits[:-1]:
                        _WS_CTR[0] += 1
                        carrier = mybir.InstEventSemaphore(
                            name=f"waitsplit_{_WS_CTR[0]}", ins=[], outs=[]
                        )
                        carrier.engine = inst.engine
                        carrier.sync_info = mybir.SyncInfo(on_wait=[w], on_update=[])
                        new.append(carrier)
                    si.on_wait = [waits[-1]]
                    changed = True
                new.append(inst)
            if changed:
                blk.instructions = new


# --------------------------------------------------------------------------
def build_nc(split_waits=True, n_o2=N_O2):
    nc = bass.Bass("TRN2", target_bir_lowering=False, debug=False,
                   num_devices=N_CORES)

    WM = nc.dram_tensor("WM", [O_LOC, D, D], BF16, kind="ExternalInput").ap()
    X1T = nc.dram_tensor("X1T", [D, BL], BF16, kind="ExternalInput").ap()
    X2T = nc.dram_tensor("X2T", [D, BL], BF16, kind="ExternalInput").ap()
    OUT = nc.dram_tensor("OUT", [B, O_LOC, L, L], BF16, kind="ExternalOutput").ap()

    with tile.TileContext(nc) as tc:
        with (
            tc.tile_pool(name="const", bufs=1) as cst,
            tc.tile_pool(name="w", bufs=3) as wpool,
            tc.tile_pool(name="ut", bufs=2) as utpool,
            tc.tile_pool(name="cs", bufs=4) as cspool,
        ):
            ctx_psA = tc.tile_pool(name="psA", bufs=2, space="PSUM")
            ctx_psB = tc.tile_pool(name="psB", bufs=2, space="PSUM")
            psA = ctx_psA.__enter__()
            psB = ctx_psB.__enter__()

            # ---- PE warm-up: dummy matmuls on a zeroed scratch tile ----------
            # They fill the startup DMA wait so the HAM activity window is
            # already open when the first real group issues.
            warm = cst.tile([128, 768], BF16, tag="warm")
            nc.vector.memset(warm[:], 0)
            wt = psA.tile([128, 1024], F32, tag="pa", name="pawarm")
            for k in range(8):
                nc.tensor.matmul(
                    wt[:, 0:512], lhsT=warm[:, 0:128], rhs=warm[:, 256:768],
                    start=(k == 0), stop=(k == 7),
                )

            # ---- resident inputs --------------------------------------------
            # X1 in 4 it-chunks: the first A group can start after chunk 0.
            X1s = cst.tile([128, 4, BL], BF16, tag="x1s")     # [i%128, it, b*256+x]
            for it in range(4):
                nc.sync.dma_start(
                    out=X1s[:, it, :], in_=X1T[it * 128:(it + 1) * 128, :])
            X2s = cst.tile([128, 4, BL], BF16, tag="x2s")     # [j%128, jt, b*256+y]

            def emit_W(o2, Ws=None, chunked=False):
                if Ws is None:
                    Ws = wpool.tile([128, 2, 4, D], BF16, tag="ws",
                                    name=f"w{o2}")   # [i%128, oi, it, j]
                if chunked:
                    # 4 x 256KB chunks (512B descriptors), oi=0 first so the
                    # first A group waits on only one chunk.
                    for oi in range(2):
                        for h in range(2):
                            nc.scalar.dma_start(
                                out=Ws[:, oi, :, h * 256:(h + 1) * 256],
                                in_=WM[2 * o2 + oi][:, h * 256:(h + 1) * 256]
                                    .rearrange("(it p) j -> p it j", p=128),
                            )
                else:
                    nc.scalar.dma_start(
                        out=Ws[:],
                        in_=WM[2 * o2:2 * o2 + 2]
                            .rearrange("oi (it p) j -> p oi it j", p=128),
                    )
                return Ws

            # ---- startup gating ---------------------------------------------
            # Only X1 + W0 may be in flight at t=0.  Everything else is held
            # back by WAW edges on dummy writes whose source (`gate`) reads
            # first X1s, then W0 -- so those triggers fire only once BOTH
            # critical startup DMAs complete.
            W0 = emit_W(0, chunked=True)
            lateW = [wpool.tile([128, 2, 4, D], BF16, tag="ws", name=f"w{i + 1}")
                     for i in range(min(3, n_o2) - 1)]
            gate = cst.tile([1, 8], BF16, tag="gate")
            nc.vector.tensor_copy(out=gate[0:1, 0:4], in_=X1s[0:1, 3, 0:4])
            nc.vector.tensor_copy(out=gate[0:1, 4:8], in_=W0[0:1, 1, 3, 508:512])
            for t_ap in ([X2s[0:1, 0, 0:4]] +
                         [w[0:1, 0, 0, 0:4] for w in lateW]):
                nc.vector.tensor_copy(out=t_ap, in_=gate[0:1, 4:8])
            wT = [W0] + lateW            # ring of 3, reused round-robin
            for i, w in enumerate(lateW):
                emit_W(i + 1, Ws=w)
            nc.sync.dma_start(
                out=X2s[:], in_=X2T.rearrange("(jt p) c -> p jt c", p=128))

            # ---- main loop over o-pairs, software-pipelined ------------------
            # All working tiles are allocated ONCE and ping-ponged manually:
            # Tile tracks byte-range WAR/WAW deps identically, but ~13 tiles
            # instead of ~150 per-iteration pool allocations means ~13 release
            # events in the epilogue ladder instead of ~150.
            paT = [psA.tile([128, 1024], F32, tag="pa", name=f"pa{i}")
                   for i in range(2)]
            pbT = [psB.tile([128, 1024], F32, tag="pb", name=f"pb{i}")
                   for i in range(2)]
            csT = [cspool.tile([128, 2, 2, 256], BF16, tag="cs", name=f"cs{i}")
                   for i in range(4)]
            utT = [utpool.tile([128, 4, 2, 2, 2, 256], BF16, tag="ut",
                               name=f"ut{i}") for i in range(2)]
            _pa_ctr = [0]
            _cs_ctr = [0]

            def emit_A(o2, Ws):
                # UT[p, jt, bp, b2, oi, x]: step B's rhs slice [oi, x] is
                # contiguous; each (oi, jt) 2-group PSUM pair evicts in one
                # 1024-wide cast.
                UT = utT[o2 % 2]
                for oi in range(2):
                    for jt in range(4):
                        pa2 = paT[_pa_ctr[0] % 2]
                        _pa_ctr[0] += 1
                        for bp in range(2):
                            for it in range(4):
                                nc.tensor.matmul(
                                    pa2[:, bp * 512:(bp + 1) * 512],
                                    lhsT=Ws[:, oi, it, jt * 128:(jt + 1) * 128],
                                    rhs=X1s[:, it, bp * 512:(bp + 1) * 512],
                                    start=(it == 0), stop=(it == 3),
                                )
                        nc.vector.tensor_copy(
                            out=UT[:, jt, :, :, oi, :], in_=pa2[:])
                return UT

            def emit_B(o2, UT):
                # out[y, (yt, oi, x)] per b; one 1024-wide cast, OUT DMAs
                # alternate between the sync and scalar HWDGE rings.  On the
                # final o-pair the eviction is split per-yt so the last OUT
                # DMA launches as soon as half the PSUM pair is drained.
                last = (o2 == n_o2 - 1)
                for b in range(B):
                    bp, b2 = divmod(b, 2)
                    pb2 = pbT[b % 2]
                    for yt in range(2):
                        for jt in range(4):
                            nc.tensor.matmul(
                                pb2[:, yt * 512:(yt + 1) * 512],
                                lhsT=X2s[:, jt, b * L + yt * 128:
                                         b * L + (yt + 1) * 128],
                                rhs=UT[:, jt, bp, b2, :, :],
                                start=(jt == 0), stop=(jt == 3),
                            )
                    # cs2 layout [oi, yt, x] so BOTH sides of the OUT DMA
                    # merge to <=3 AP dims; the cast writes through a
                    # [yt, oi, x]-ordered view to match pb2's bank layout.
                    cs2 = csT[_cs_ctr[0] % 4]
                    _cs_ctr[0] += 1
                    eng = nc.sync if b % 2 == 0 else nc.scalar
                    if last and b >= 2:
                        for yt in range(2):
                            nc.vector.tensor_copy(
                                out=cs2[:, :, yt, :],
                                in_=pb2[:, yt * 512:(yt + 1) * 512])
                            eng.dma_start(
                                out=OUT[b, 2 * o2:2 * o2 + 2,
                                        yt * 128:(yt + 1) * 128]
                                    .rearrange("og p x -> p og x"),
                                in_=cs2[:, :, yt, :],
                            )
                    else:
                        nc.vector.tensor_copy(
                            out=cs2[:].rearrange("p oi yt x -> p yt oi x"),
                            in_=pb2[:])
                        eng.dma_start(
                            out=OUT[b, 2 * o2:2 * o2 + 2]
                                .rearrange("og (yt p) x -> p og yt x", p=128),
                            in_=cs2[:],
                        )

            UT_prev = emit_A(0, wT[0])
            for o2 in range(n_o2):
                if o2 + 3 < n_o2:
                    emit_W(o2 + 3, Ws=wT[(o2 + 3) % 3])
                UT_next = (emit_A(o2 + 1, wT[(o2 + 1) % 3])
                           if o2 + 1 < n_o2 else None)
                emit_B(o2, UT_prev)
                UT_prev = UT_next

            ctx_psB.__exit__(None, None, None)
            ctx_psA.__exit__(None, None, None)

    if split_waits:
        _split_multi_waits(nc)
    return nc


_NC_CACHE = None


def _get_nc():
    global _NC_CACHE
    if _NC_CACHE is None:
        _NC_CACHE = build_nc()
    return _NC_CACHE


# host-side rank-2 terms, set by _prep_inputs, consumed by _assemble
_HOST_CTX = {}


def _prep_inputs(x1, x2, bw, W_bil, W_lin, b_lin):
    """Host-side glue: softmax of the 2-vector, per-core weight slicing, and
    the cheap rank-2 D-terms (D1 = x1@G+g0, D2 = x2@V) kept for _assemble."""
    x1 = np.asarray(x1, np.float32)
    x2 = np.asarray(x2, np.float32)
    bw = np.asarray(bw, np.float64)
    W_bil = np.asarray(W_bil, np.float32)
    W_lin = np.asarray(W_lin, np.float32)
    b_lin = np.asarray(b_lin, np.float32)

    e = np.exp(bw - bw.max())
    bwn = (e / e.sum()).astype(np.float32)
    bwn0, bwn1 = float(bwn[0]), float(bwn[1])

    x1T = np.ascontiguousarray(
        x1.transpose(2, 0, 1).reshape(D, BL).astype(ml_dtypes.bfloat16))
    x2T = np.ascontiguousarray(
        x2.transpose(2, 0, 1).reshape(D, BL).astype(ml_dtypes.bfloat16))

    # rank-2 terms over the FULL O, added on the host in _assemble
    G = bwn0 * W_bil[:, :D, D].T + bwn1 * W_lin[:D]        # [D, O]
    V = bwn0 * W_bil[:, D, :D].T + bwn1 * W_lin[D:]        # [D, O]
    g0 = bwn0 * W_bil[:, D, D] + bwn1 * b_lin              # [O]
    _HOST_CTX["D1"] = np.einsum('bxd,do->bxo', x1, G) + g0  # [B, L, O]
    _HOST_CTX["D2"] = np.einsum('byd,do->byo', x2, V)       # [B, L, O]

    in_maps = []
    for c in range(N_CORES):
        o_sl = slice(c * O_LOC, (c + 1) * O_LOC)
        WMv = np.ascontiguousarray(
            (bwn0 * W_bil[o_sl, :D, :D]).astype(ml_dtypes.bfloat16))
        in_maps.append({"WM": WMv, "X1T": x1T, "X2T": x2T})
    return in_maps


def _assemble(results):
    out = np.empty((B, L, L, O), np.float32)
    for c in range(N_CORES):
        # per-core OUT is [b, o_local, y, x] bf16 -> full is [b, x, y, o]
        out[:, :, :, c * O_LOC:(c + 1) * O_LOC] = \
            results[c]["OUT"].transpose(0, 3, 2, 1).astype(np.float32)
    out += _HOST_CTX["D1"][:, :, None, :]
    out += _HOST_CTX["D2"][:, None, :, :]
    return out


def kernel(**inputs):
    in_maps = _prep_inputs(**inputs)
    nc = _get_nc()
    res = run_bass_kernel_spmd(nc, in_maps, list(range(N_CORES)))
    return _assemble(res.results)


# revision 9
# speedup vs baseline: 1.0141x; 1.0141x over previous
"""Biaffine kernel for Trainium2, 8-core SPMD.

Math (reference):
    out[b,x,y,o] = bwn0 * sum_{i,j<=512} x1b[b,x,i] W_bil[o,i,j] x2b[b,y,j]
                 + bwn1 * (x1@W_lin[:512] [b,x,o] + x2@W_lin[512:] [b,y,o] + b_lin[o])
    with x1b/x2b = x append-ones, bwn = softmax(bw).

Decomposition (exact):
    out[b,x,y,o] = sum_{j<512} x2[b,y,j] * UT[b,o][j,x]      (step B, PE)
                 + D1[b,x,o] + D2[b,y,o]                      (rank-2, added on HOST)
    UT[b,o][j,x] = sum_{i<512} (bwn0*W_bil[o,i,j]) * x1[b,x,i]   (step A, PE)
    The device stream is pure 512-deep bf16 matmuls; the cheap rank-2 terms
    (x1@G+g0, x2@V) and the final broadcast-add run on the host, keeping the
    DVE to plain PSUM->SBUF casts that never gate the PE.

Sharding: tensor-parallel over O (128 output channels -> 16 per core).

Schedule notes:
  - PSUM tiles are 2-bank pairs ([128,1024] f32); each pair holds two 4-MM
    accumulation groups and is evicted by ONE 1024-wide DVE cast, halving
    the per-group eviction overhead (psA 2x2 + psB 2x2 = 8 banks).
  - Startup: X1 arrives in 4 it-chunks (2KB descriptors) and W0 in 4
    256KB chunks so the first matmul group only waits on ~512KB; all other
    loads (X2, W1, W2) are gated on BOTH X1 and W0 completing so they can't
    steal HBM bandwidth from the critical path (the v1 gate released them
    on X1 alone, which starved W0 for >10us).
  - 8 dummy matmuls on a zeroed scratch tile run during the startup DMA
    wait so the PE's HAM activity window is already warm when real work
    arrives.
  - OUT is written bf16 (host upcasts), halving output DMA traffic.
"""

import ml_dtypes
import numpy as np

import concourse.bass as bass
import concourse.mybir as mybir
import concourse.tile as tile
from concourse.bass_utils import run_bass_kernel_spmd

B, L, D, O = 4, 256, 512, 128
BL = B * L
N_CORES = 8
O_LOC = O // N_CORES          # 16 output channels per core
N_O2 = O_LOC // 2             # 8 o-pairs per core
F32 = mybir.dt.float32
BF16 = mybir.dt.bfloat16


# --------------------------------------------------------------------------
# Workaround: this container's walrus build accepts only ONE sync wait per
# instruction ("Too many sync wait commands").  Tile's wait assignment can
# attach several.  Post-pass: hoist extra waits onto InstEventSemaphore
# wait-carriers inserted immediately before the instruction on the same
# engine stream (same stall point, identical semantics).
_WS_CTR = [0]


def _split_multi_waits(nc):
    for f in nc.m.functions:
        for blk in f.blocks:
            insts = blk.instructions
            new = []
            changed = False
            for inst in insts:
                si = inst.sync_info
                waits = list(si.on_wait) if (si and si.on_wait) else []
                if len(waits) > 1:
                    for w in waits[:-1]:
                        _WS_CTR[0] += 1
                        carrier = mybir.InstEventSemaphore(
                            name=f"waitsplit_{_WS_CTR[0]}", ins=[], outs=[]
                        )
                        carrier.engine = inst.engine
                        carrier.sync_info = mybir.SyncInfo(on_wait=[w], on_update=[])
                        new.append(carrier)
                    si.on_wait = [waits[-1]]
                    changed = True
                new.append(inst)
            if changed:
                blk.instructions = new


# --------------------------------------------------------------------------
def build_nc(split_waits=True, n_o2=N_O2):
    nc = bass.Bass("TRN2", target_bir_lowering=False, debug=False,
                   num_devices=N_CORES)

    WM = nc.dram_tensor("WM", [O_LOC, D, D], BF16, kind="ExternalInput").ap()
    X1T = nc.dram_tensor("X1T", [D, BL], BF16, kind="ExternalInput").ap()
    X2T = nc.dram_tensor("X2T", [D, BL], BF16, kind="ExternalInput").ap()
    OUT = nc.dram_tensor("OUT", [B, O_LOC, L, L], BF16, kind="ExternalOutput").ap()

    with tile.TileContext(nc) as tc:
        with (
            tc.tile_pool(name="const", bufs=1) as cst,
            tc.tile_pool(name="w", bufs=3) as wpool,
            tc.tile_pool(name="ut", bufs=2) as utpool,
            tc.tile_pool(name="cs", bufs=4) as cspool,
        ):
            ctx_psA = tc.tile_pool(name="psA", bufs=2, space="PSUM")
            ctx_psB = tc.tile_pool(name="psB", bufs=2, space="PSUM")
            psA = ctx_psA.__enter__()
            psB = ctx_psB.__enter__()

            # ---- PE warm-up: dummy matmuls on a zeroed scratch tile ----------
            # They fill the startup DMA wait so the HAM activity window is
            # already open when the first real group issues.
            warm = cst.tile([128, 768], BF16, tag="warm")
            nc.vector.memset(warm[:], 0)
            wt = psA.tile([128, 1024], F32, tag="pa", name="pawarm")
            for k in range(6):
                nc.tensor.matmul(
                    wt[:, 0:512], lhsT=warm[:, 0:128], rhs=warm[:, 256:768],
                    start=(k == 0), stop=(k == 5),
                )

            # ---- resident inputs --------------------------------------------
            # X1 in 8 bp-major chunks: the first A group (bp=0) only needs the
            # first four 128KB chunks, so real matmuls start ~1.5us earlier.
            X1s = cst.tile([128, 4, BL], BF16, tag="x1s")     # [i%128, it, b*256+x]
            for bp in range(2):
                for it in range(4):
                    nc.sync.dma_start(
                        out=X1s[:, it, bp * 512:(bp + 1) * 512],
                        in_=X1T[it * 128:(it + 1) * 128,
                                bp * 512:(bp + 1) * 512])
            X2s = cst.tile([128, 4, BL], BF16, tag="x2s")     # [j%128, jt, b*256+y]

            def emit_W(o2, Ws=None, chunked=False):
                if Ws is None:
                    Ws = wpool.tile([128, 2, 4, D], BF16, tag="ws",
                                    name=f"w{o2}")   # [i%128, oi, it, j]
                if chunked:
                    # 4 x 256KB chunks (512B descriptors), oi=0 first so the
                    # first A group waits on only one chunk.
                    for oi in range(2):
                        for h in range(2):
                            nc.scalar.dma_start(
                                out=Ws[:, oi, :, h * 256:(h + 1) * 256],
                                in_=WM[2 * o2 + oi][:, h * 256:(h + 1) * 256]
                                    .rearrange("(it p) j -> p it j", p=128),
                            )
                else:
                    nc.scalar.dma_start(
                        out=Ws[:],
                        in_=WM[2 * o2:2 * o2 + 2]
                            .rearrange("oi (it p) j -> p oi it j", p=128),
                    )
                return Ws

            # ---- startup gating ---------------------------------------------
            # Only X1 + W0 may be in flight at t=0.  Everything else is held
            # back by WAW edges on dummy writes whose source (`gate`) reads
            # first X1s, then W0 -- so those triggers fire only once BOTH
            # critical startup DMAs complete.
            W0 = emit_W(0, chunked=True)
            lateW = [wpool.tile([128, 2, 4, D], BF16, tag="ws", name=f"w{i + 1}")
                     for i in range(min(3, n_o2) - 1)]
            gate = cst.tile([1, 8], BF16, tag="gate")
            nc.vector.tensor_copy(out=gate[0:1, 0:4], in_=X1s[0:1, 3, 1020:1024])
            nc.vector.tensor_copy(out=gate[0:1, 4:8], in_=W0[0:1, 1, 3, 508:512])
            for t_ap in ([X2s[0:1, 0, 0:4]] +
                         [w[0:1, 0, 0, 0:4] for w in lateW]):
                nc.vector.tensor_copy(out=t_ap, in_=gate[0:1, 4:8])
            wT = [W0] + lateW            # ring of 3, reused round-robin
            for i, w in enumerate(lateW):
                emit_W(i + 1, Ws=w)
            nc.sync.dma_start(
                out=X2s[:], in_=X2T.rearrange("(jt p) c -> p jt c", p=128))

            # ---- main loop over o-pairs, software-pipelined ------------------
            # All working tiles are allocated ONCE and ping-ponged manually:
            # Tile tracks byte-range WAR/WAW deps identically, but ~13 tiles
            # instead of ~150 per-iteration pool allocations means ~13 release
            # events in the epilogue ladder instead of ~150.
            paT = [psA.tile([128, 1024], F32, tag="pa", name=f"pa{i}")
                   for i in range(2)]
            pbT = [psB.tile([128, 1024], F32, tag="pb", name=f"pb{i}")
                   for i in range(2)]
            csT = [cspool.tile([128, 2, 2, 256], BF16, tag="cs", name=f"cs{i}")
                   for i in range(4)]
            utT = [utpool.tile([128, 4, 2, 2, 2, 256], BF16, tag="ut",
                               name=f"ut{i}") for i in range(2)]
            _pa_ctr = [0]
            _cs_ctr = [0]

            def emit_A(o2, Ws):
                # UT[p, jt, bp, b2, oi, x]: step B's rhs slice [oi, x] is
                # contiguous; each (oi, jt) 2-group PSUM pair evicts in one
                # 1024-wide cast.
                UT = utT[o2 % 2]
                for oi in range(2):
                    for jt in range(4):
                        pa2 = paT[_pa_ctr[0] % 2]
                        _pa_ctr[0] += 1
                        for bp in range(2):
                            for it in range(4):
                                nc.tensor.matmul(
                                    pa2[:, bp * 512:(bp + 1) * 512],
                                    lhsT=Ws[:, oi, it, jt * 128:(jt + 1) * 128],
                                    rhs=X1s[:, it, bp * 512:(bp + 1) * 512],
                                    start=(it == 0), stop=(it == 3),
                                )
                        nc.vector.tensor_copy(
                            out=UT[:, jt, :, :, oi, :], in_=pa2[:])
                return UT

            def emit_B(o2, UT):
                # out[y, (yt, oi, x)] per b; one 1024-wide cast, OUT DMAs
                # alternate between the sync and scalar HWDGE rings.  On the
                # final o-pair the eviction is split per-yt so the last OUT
                # DMA launches as soon as half the PSUM pair is drained.
                last = (o2 == n_o2 - 1)
                for b in range(B):
                    bp, b2 = divmod(b, 2)
                    pb2 = pbT[b % 2]
                    for yt in range(2):
                        for jt in range(4):
                            nc.tensor.matmul(
                                pb2[:, yt * 512:(yt + 1) * 512],
                                lhsT=X2s[:, jt, b * L + yt * 128:
                                         b * L + (yt + 1) * 128],
                                rhs=UT[:, jt, bp, b2, :, :],
                                start=(jt == 0), stop=(jt == 3),
                            )
                    # cs2 layout [oi, yt, x] so BOTH sides of the OUT DMA
                    # merge to <=3 AP dims; the cast writes through a
                    # [yt, oi, x]-ordered view to match pb2's bank layout.
                    cs2 = csT[_cs_ctr[0] % 4]
                    _cs_ctr[0] += 1
                    eng = nc.sync if b % 2 == 0 else nc.scalar
                    if last and b >= 2:
                        for yt in range(2):
                            nc.vector.tensor_copy(
                                out=cs2[:, :, yt, :],
                                in_=pb2[:, yt * 512:(yt + 1) * 512])
                            eng.dma_start(
                                out=OUT[b, 2 * o2:2 * o2 + 2,
                                        yt * 128:(yt + 1) * 128]
                                    .rearrange("og p x -> p og x"),
                                in_=cs2[:, :, yt, :],
                            )
                    else:
                        nc.vector.tensor_copy(
                            out=cs2[:].rearrange("p oi yt x -> p yt oi x"),
                            in_=pb2[:])
                        eng.dma_start(
                            out=OUT[b, 2 * o2:2 * o2 + 2]
                                .rearrange("og (yt p) x -> p og yt x", p=128),
                            in_=cs2[:],
                        )

            UT_prev = emit_A(0, wT[0])
            for o2 in range(n_o2):
                if o2 + 3 < n_o2:
                    emit_W(o2 + 3, Ws=wT[(o2 + 3) % 3])
                UT_next = (emit_A(o2 + 1, wT[(o2 + 1) % 3])
                           if o2 + 1 < n_o2 else None)
                emit_B(o2, UT_prev)
                UT_prev = UT_next

            ctx_psB.__exit__(None, None, None)
            ctx_psA.__exit__(None, None, None)

    if split_waits:
        _split_multi_waits(nc)
    return nc


_NC_CACHE = None


def _get_nc():
    global _NC_CACHE
    if _NC_CACHE is None:
        _NC_CACHE = build_nc()
    return _NC_CACHE


# host-side rank-2 terms, set by _prep_inputs, consumed by _assemble
_HOST_CTX = {}


def _prep_inputs(x1, x2, bw, W_bil, W_lin, b_lin):
    """Host-side glue: softmax of the 2-vector, per-core weight slicing, and
    the cheap rank-2 D-terms (D1 = x1@G+g0, D2 = x2@V) kept for _assemble."""
    x1 = np.asarray(x1, np.float32)
    x2 = np.asarray(x2, np.float32)
    bw = np.asarray(bw, np.float64)
    W_bil = np.asarray(W_bil, np.float32)
    W_lin = np.asarray(W_lin, np.float32)
    b_lin = np.asarray(b_lin, np.float32)

    e = np.exp(bw - bw.max())
    bwn = (e / e.sum()).astype(np.float32)
    bwn0, bwn1 = float(bwn[0]), float(bwn[1])

    x1T = np.ascontiguousarray(
        x1.transpose(2, 0, 1).reshape(D, BL).astype(ml_dtypes.bfloat16))
    x2T = np.ascontiguousarray(
        x2.transpose(2, 0, 1).reshape(D, BL).astype(ml_dtypes.bfloat16))

    # rank-2 terms over the FULL O, added on the host in _assemble
    G = bwn0 * W_bil[:, :D, D].T + bwn1 * W_lin[:D]        # [D, O]
    V = bwn0 * W_bil[:, D, :D].T + bwn1 * W_lin[D:]        # [D, O]
    g0 = bwn0 * W_bil[:, D, D] + bwn1 * b_lin              # [O]
    _HOST_CTX["D1"] = np.einsum('bxd,do->bxo', x1, G) + g0  # [B, L, O]
    _HOST_CTX["D2"] = np.einsum('byd,do->byo', x2, V)       # [B, L, O]

    in_maps = []
    for c in range(N_CORES):
        o_sl = slice(c * O_LOC, (c + 1) * O_LOC)
        WMv = np.ascontiguousarray(
            (bwn0 * W_bil[o_sl, :D, :D]).astype(ml_dtypes.bfloat16))
        in_maps.append({"WM": WMv, "X1T": x1T, "X2T": x2T})
    return in_maps


def _assemble(results):
    out = np.empty((B, L, L, O), np.float32)
    for c in range(N_CORES):
        # per-core OUT is [b, o_local, y, x] bf16 -> full is [b, x, y, o]
        out[:, :, :, c * O_LOC:(c + 1) * O_LOC] = \
            results[c]["OUT"].transpose(0, 3, 2, 1).astype(np.float32)
    out += _HOST_CTX["D1"][:, :, None, :]
    out += _HOST_CTX["D2"][:, None, :, :]
    return out


def kernel(**inputs):
    in_maps = _prep_inputs(**inputs)
    nc = _get_nc()
    res = run_bass_kernel_spmd(nc, in_maps, list(range(N_CORES)))
    return _assemble(res.results)
